# revision 1
# baseline (speedup 1.0000x reference)
"""Trainium2 Bass kernel for nn_Attention_11055245820093.

Swin-style attention block: qkv proj -> per-head scaled dot-product attention
with 2D relative position bias (CLS zero-padded), per-head softplus temperature,
patch-diagonal mask -> proj.

Strategy: data-parallel over batch B=64 across 8 NeuronCores (8 batches/core).
All compute per core runs in a "transposed" layout (channels on partitions,
tokens on the free dim) so no on-device transposes are needed:
  - QK^T projection computed in out^T (c, t) layout
  - V computed in (t, c) layout directly (lhsT = x^T)
  - S^T(j,i) = K^T.T @ Q^T per (batch,head); scale/temp folded into the q
    weights; rel-pos bias/mask applied multiplicatively via a host-exp'd table
  - AV uses V(j,d) as lhsT directly; softmax denominators from a ones-column
    matmul; division via a PE-broadcast of the reciprocal row
  - proj consumes attn_out^T directly; v-bias and proj bias fold into a
    host-side constant added after gather.
Matmuls in bf16 (fp32 PSUM accumulate); softmax math in fp32.
"""

import os
import sys

sys.path.insert(0, "/opt/trn_rl_repo")
os.environ.setdefault("MYCRO_LOCAL_CACHE", "1")

import numpy as np
import ml_dtypes

BF16 = ml_dtypes.bfloat16

# Problem constants (hardcoded per contract)
B, N, C, H, D = 64, 197, 768, 12, 64
NCORES = 8
BPC = B // NCORES          # 8 batches per core
T = BPC * N                # 1576 tokens per core
KT = C // 128              # 6 contraction tiles of 128
NT = 4                     # token n-tiles
TN = T // NT               # 394 tokens per n-tile
SCALE = D ** -0.5

_CACHE = {}

TRACE = False
LAST_RESULTS = None


def _build(finalize=True):
    import concourse.bass as bass
    import concourse.tile as tile
    from concourse import bacc, mybir

    dt = mybir.dt
    f32, bf16 = dt.float32, dt.bfloat16
    AF = mybir.ActivationFunctionType
    OP = mybir.AluOpType

    nc = bacc.Bacc("TRN2", target_bir_lowering=False, debug=False)

    xT = nc.dram_tensor("xT", [KT, 128, T], bf16, kind="ExternalInput").ap()
    wqk = nc.dram_tensor("wqk", [KT, 128, 2 * C], bf16, kind="ExternalInput").ap()
    wv = nc.dram_tensor("wv", [KT, 128, C], bf16, kind="ExternalInput").ap()
    wpj = nc.dram_tensor("wpj", [KT, 128, C], bf16, kind="ExternalInput").ap()
    bT = nc.dram_tensor("bT", [KT, N, 2 * N], bf16, kind="ExternalInput").ap()
    bqk = nc.dram_tensor("bqk", [128, 2 * KT], f32, kind="ExternalInput").ap()
    outT = nc.dram_tensor("outT", [KT, 128, T], f32, kind="ExternalOutput").ap()

    JROWS = (128, N - 128)  # 128, 69

    with tile.TileContext(nc) as tc:
        from contextlib import ExitStack

        with ExitStack() as ctx:
            cp = ctx.enter_context(tc.tile_pool(name="consts", bufs=1))
            psA = ctx.enter_context(tc.tile_pool(name="psA", bufs=2, space="PSUM"))
            psS = ctx.enter_context(tc.tile_pool(name="psS", bufs=2, space="PSUM"))
            psO = ctx.enter_context(tc.tile_pool(name="psO", bufs=1, space="PSUM"))
            wp = ctx.enter_context(tc.tile_pool(name="work", bufs=2))

            # ---- persistent SBUF tiles; DMAs in consumption order ----
            x_sb = []
            for k in range(KT):
                t_ = cp.tile([128, T], bf16, name=f"x{k}", tag=f"x{k}")
                nc.sync.dma_start(out=t_[:], in_=xT[k])
                x_sb.append(t_)
            wv_sb = []
            for k in range(KT):
                t_ = cp.tile([128, C], bf16, name=f"wv{k}", tag=f"wv{k}")
                nc.gpsimd.dma_start(out=t_[:], in_=wv[k])
                wv_sb.append(t_)
            wqk_sb = []
            for k in range(KT):
                t_ = cp.tile([128, 2 * C], bf16, name=f"wqk{k}", tag=f"wqk{k}")
                nc.gpsimd.dma_start(out=t_[:], in_=wqk[k])
                wqk_sb.append(t_)
            bqk_sb = cp.tile([128, 2 * KT], f32, name="bqk", tag="bqk")
            nc.gpsimd.dma_start(out=bqk_sb[:], in_=bqk[:])
            # bias (hp, jt): both heads of the pair side by side (rows, 2N);
            # loads deferred into the hp loop. proj weights load before D.
            bias_sb = {}
            for hp in range(KT):
                for jt, rows in enumerate(JROWS):
                    bias_sb[(hp, jt)] = cp.tile(
                        [rows, 2 * N], bf16, name=f"bias{hp}_{jt}", tag=f"bias{hp}_{jt}"
                    )
            wpj_sb = [
                cp.tile([128, C], bf16, name=f"wpj{k}", tag=f"wpj{k}")
                for k in range(KT)
            ]

            # qk_sb[0:6] = Q^T tiles (c=0..767), qk_sb[6:12] = K^T tiles
            qk_sb = [
                cp.tile([128, T], bf16, name=f"qk{m}", tag=f"qk{m}")
                for m in range(2 * KT)
            ]
            # V per (batch, jt): plain (rows, 768), head h at cols h*64..
            v_sb = {}
            for b in range(BPC):
                for jt, rows in enumerate(JROWS):
                    v_sb[(b, jt)] = cp.tile(
                        [rows, C], bf16, name=f"v{b}_{jt}", tag=f"v{b}_{jt}"
                    )
            attn_sb = [
                cp.tile([128, T], bf16, name=f"at{m}", tag=f"at{m}") for m in range(KT)
            ]

            # ones columns for the softmax-denominator matmuls, and a ones row
            # for the reciprocal broadcast matmul
            onesc = {}
            for jt, rows in enumerate(JROWS):
                oc = cp.tile([rows, 1], bf16, name=f"onesc{jt}", tag=f"onesc{jt}")
                nc.vector.memset(oc[:], 1.0)
                onesc[jt] = oc
            ones_r = cp.tile([1, 128], bf16, name="ones_r", tag="ones_r")
            nc.vector.memset(ones_r[:], 1.0)

            # ---- Phase B: V in (t, c) layout ----
            for b in range(BPC):
                for jt, rows in enumerate(JROWS):
                    for n2 in range(2):
                        psv = psA.tile([128, TN], f32, tag="psA")
                        for k in range(KT):
                            nc.tensor.matmul(
                                psv[0:rows, 0 : C // 2],
                                x_sb[k][:, b * N + jt * 128 : b * N + jt * 128 + rows],
                                wv_sb[k][:, n2 * (C // 2) : (n2 + 1) * (C // 2)],
                                start=(k == 0),
                                stop=(k == KT - 1),
                            )
                        # v-bias folds into the host-side output constant
                        nc.vector.tensor_copy(
                            v_sb[(b, jt)][0:rows, n2 * (C // 2) : (n2 + 1) * (C // 2)],
                            psv[0:rows, 0 : C // 2],
                        )

            # ---- Phase A (by head-pair) interleaved with Phase C ----
            def emit_proj_tile(mt):
                for nt in range(NT):
                    ps = psA.tile([128, TN], f32, tag="psA")
                    for k in range(KT):
                        nc.tensor.matmul(
                            ps[:],
                            wqk_sb[k][:, mt * 128 : (mt + 1) * 128],
                            x_sb[k][:, nt * TN : (nt + 1) * TN],
                            start=(k == 0),
                            stop=(k == KT - 1),
                        )
                    nc.scalar.activation(
                        qk_sb[mt][:, nt * TN : (nt + 1) * TN],
                        ps[:],
                        AF.Identity,
                        bias=bqk_sb[:, mt : mt + 1],
                    )

            for hp in range(KT):
                for jt, rows in enumerate(JROWS):
                    nc.sync.dma_start(
                        out=bias_sb[(hp, jt)][:],
                        in_=bT[hp, jt * 128 : jt * 128 + rows, :],
                    )
                emit_proj_tile(hp)          # Q^T tile for this head pair
                emit_proj_tile(KT + hp)     # K^T tile
                for b in range(BPC):
                    e_tiles = {}
                    # sums for both heads side by side: (1, 2N)
                    s_ps = psO.tile([1, 2 * N], f32, tag="psSum", bufs=1)
                    for hh in range(2):
                        h = 2 * hp + hh
                        base = 64 * hh
                        for jt, rows in enumerate(JROWS):
                            ps = psS.tile([128, N], f32, tag="psS")
                            nc.tensor.matmul(
                                ps[0:rows, :],
                                qk_sb[KT + hp][
                                    base : base + 64,
                                    b * N + jt * 128 : b * N + jt * 128 + rows,
                                ],
                                qk_sb[hp][base : base + 64, b * N : (b + 1) * N],
                                start=True,
                                stop=True,
                            )
                            eu = wp.tile([128, N], bf16, tag=f"eu{hh}{jt}", bufs=2)
                            nc.scalar.activation(eu[0:rows, :], ps[0:rows, :], AF.Exp)
                            # multiplicative rel-pos bias (exp'd on host)
                            e = wp.tile([128, N], bf16, tag=f"e{hh}{jt}", bufs=2)
                            nc.vector.tensor_mul(
                                e[0:rows, :],
                                eu[0:rows, :],
                                bias_sb[(hp, jt)][
                                    0:rows, hh * N : (hh + 1) * N
                                ],
                            )
                            e_tiles[(hh, jt)] = e
                        # softmax denominators for this head
                        for jt, rows in enumerate(JROWS):
                            nc.tensor.matmul(
                                s_ps[0:1, hh * N : (hh + 1) * N],
                                onesc[jt][0:rows, :],
                                e_tiles[(hh, jt)][0:rows, :],
                                start=(jt == 0),
                                stop=(jt == 1),
                            )
                    r2 = wp.tile([1, 2 * N], bf16, tag="r2", bufs=4)
                    with nc.allow_low_precision(
                        reason="softmax denom reciprocal in bf16 for PE broadcast"
                    ):
                        nc.vector.reciprocal(r2[:], s_ps[:])
                    # broadcast 1/s to all 128 partitions (both heads' halves)
                    rbp = psO.tile([128, 2 * N], f32, tag="psO")
                    nc.tensor.matmul(rbp[:], ones_r[:], r2[:], start=True, stop=True)
                    rb = wp.tile([128, 2 * N], bf16, tag="rb", bufs=2)
                    nc.scalar.activation(rb[:], rbp[:], AF.Copy)
                    for hh in range(2):
                        h = 2 * hp + hh
                        base = 64 * hh
                        po = psS.tile([128, 512], f32, tag="psPo", bufs=2)
                        for jt, rows in enumerate(JROWS):
                            nc.tensor.matmul(
                                po[base : base + 64, 0:N],
                                v_sb[(b, jt)][0:rows, h * 64 : (h + 1) * 64],
                                e_tiles[(hh, jt)][0:rows, :],
                                start=(jt == 0),
                                stop=(jt == 1),
                            )
                        nc.vector.tensor_mul(
                            attn_sb[hp][base : base + 64, b * N : (b + 1) * N],
                            po[base : base + 64, 0:N],
                            rb[base : base + 64, hh * N : (hh + 1) * N],
                        )

            # ---- Phase D: proj -> out^T(c,t); proj bias added on host ----
            for k in range(KT):
                nc.sync.dma_start(out=wpj_sb[k][:], in_=wpj[k])
            for mt in range(KT):
                for nt in range(NT):
                    ps = psA.tile([128, TN], f32, tag="psA")
                    for k in range(KT):
                        nc.tensor.matmul(
                            ps[:],
                            wpj_sb[k][:, mt * 128 : (mt + 1) * 128],
                            attn_sb[k][:, nt * TN : (nt + 1) * TN],
                            start=(k == 0),
                            stop=(k == KT - 1),
                        )
                    ot = wp.tile([128, TN], f32, tag="ot", bufs=3)
                    nc.scalar.activation(ot[:], ps[:], AF.Copy)
                    nc.sync.dma_start(
                        out=outT[mt, :, nt * TN : (nt + 1) * TN], in_=ot[:]
                    )

    if finalize:
        nc.finalize()
    return nc


def _host_prep(x, qkv_w, qkv_b, proj_w, proj_b, rel_table, log_temp, rel_index):
    """Build the per-core input maps (host-side layout prep only)."""
    x = np.asarray(x, np.float32)
    qkv_w = np.asarray(qkv_w, np.float32)
    qkv_b = np.asarray(qkv_b, np.float32)
    proj_w = np.asarray(proj_w, np.float32)
    rel_table = np.asarray(rel_table, np.float32)
    log_temp = np.asarray(log_temp, np.float32)
    rel_index = np.asarray(rel_index)

    temp = np.log1p(np.exp(log_temp.astype(np.float64))).astype(np.float32)  # softplus
    alpha = (SCALE / temp).astype(np.float32)         # (H,) folded into q
    alpha_c = np.repeat(alpha, D)                     # (768,)

    wqkT = qkv_w[0 : 2 * C].T.copy()                  # (768, 1536)
    wqkT[:, 0:C] *= alpha_c[None, :]
    wqk_np = wqkT.reshape(KT, 128, 2 * C).astype(BF16)

    wv_np = qkv_w[2 * C : 3 * C].T.reshape(KT, 128, C).astype(BF16)
    wpj_np = proj_w.T.reshape(KT, 128, C).astype(BF16)

    bq = qkv_b[0:C] * alpha_c
    bk = qkv_b[C : 2 * C]
    bqk_np = np.concatenate([bq, bk]).reshape(2 * KT, 128).T.copy().astype(np.float32)

    # multiplicative bias table: exp((relpos bias)/temp), diag -> 0, CLS -> 1,
    # transposed to (j, i)
    rpb = rel_table[rel_index]                        # (196, 196, H)
    bias = np.zeros((H, N, N), np.float32)
    bias[:, 1:, 1:] = rpb.transpose(2, 0, 1) / temp[:, None, None]
    ebias = np.exp(bias)
    idx = np.arange(1, N)
    ebias[:, idx, idx] = 0.0
    ebT = ebias.transpose(0, 2, 1)                    # (H, j, i)
    # paired layout: (KT, j, 2N) = heads 2hp | 2hp+1 side by side
    bT_np = (
        ebT.reshape(KT, 2, N, N).transpose(0, 2, 1, 3).reshape(KT, N, 2 * N)
    ).astype(BF16).copy()

    in_maps = []
    for c in range(NCORES):
        xc = x[c * BPC : (c + 1) * BPC].reshape(T, C).T  # (768, T)
        xT_np = xc.reshape(KT, 128, T).astype(BF16)
        in_maps.append(
            {
                "xT": xT_np,
                "wqk": wqk_np,
                "wv": wv_np,
                "wpj": wpj_np,
                "bT": bT_np,
                "bqk": bqk_np,
            }
        )
    return in_maps


def kernel(**inputs) -> np.ndarray:
    global LAST_RESULTS
    from concourse.bass_utils import run_bass_kernel_spmd

    if "nc" not in _CACHE:
        _CACHE["nc"] = _build()
    nc = _CACHE["nc"]

    in_maps = _host_prep(**inputs)
    try:
        res = run_bass_kernel_spmd(
            nc, in_maps, core_ids=list(range(NCORES)), trace=TRACE
        )
    except ModuleNotFoundError:
        res = run_bass_kernel_spmd(
            nc, in_maps, core_ids=list(range(NCORES)), trace=False
        )
    LAST_RESULTS = res

    # v-bias rides through attention unchanged (rows of attn sum to 1), so
    # its proj image folds into the constant output bias added here
    proj_b = np.asarray(inputs["proj_b"], np.float32)
    proj_w = np.asarray(inputs["proj_w"], np.float32)
    bv = np.asarray(inputs["qkv_b"], np.float32)[2 * C : 3 * C]
    b_eff = proj_b + proj_w @ bv
    outs = []
    for c in range(NCORES):
        oT = np.asarray(res.results[c]["outT"], np.float32).reshape(C, T)
        outs.append(oT.T.reshape(BPC, N, C))
    out = np.concatenate(outs, axis=0) + b_eff[None, None, :]
    return out.astype(np.float32)



# revision 2
# speedup vs baseline: 1.3034x; 1.3034x over previous
"""Trainium2 Bass kernel for nn_Attention_11055245820093.

Swin-style attention block: qkv proj -> per-head scaled dot-product attention
with 2D relative position bias (CLS zero-padded), per-head softplus temperature,
patch-diagonal mask -> proj.

Strategy: data-parallel over batch B=64 across 8 NeuronCores (8 batches/core).
All compute per core runs in a "transposed" layout (channels on partitions,
tokens on the free dim) so no on-device transposes are needed:
  - QK^T projection computed in out^T (c, t) layout
  - V computed in (t, c) layout directly (lhsT = x^T)
  - S^T(j,i) = K^T.T @ Q^T per (batch,head); scale/temp folded into the q
    weights; rel-pos bias/mask applied multiplicatively via a host-exp'd table
  - AV uses V(j,d) as lhsT directly; softmax denominators from a ones-column
    matmul; division via a PE-broadcast of the reciprocal row
  - proj consumes attn_out^T directly; v-bias and proj bias fold into a
    host-side constant added after gather.
Matmuls in bf16 (fp32 PSUM accumulate); softmax math in fp32.
"""

import os
import sys

sys.path.insert(0, "/opt/trn_rl_repo")
os.environ.setdefault("MYCRO_LOCAL_CACHE", "1")

import numpy as np
import ml_dtypes

BF16 = ml_dtypes.bfloat16

# Problem constants (hardcoded per contract)
B, N, C, H, D = 64, 197, 768, 12, 64
NCORES = 8
BPC = B // NCORES          # 8 batches per core
T = BPC * N                # 1576 tokens per core
KT = C // 128              # 6 contraction tiles of 128
NT = 4                     # token n-tiles
TN = T // NT               # 394 tokens per n-tile
SCALE = D ** -0.5

_CACHE = {}

TRACE = False
LAST_RESULTS = None


def _build(finalize=True):
    import concourse.bass as bass
    import concourse.tile as tile
    from concourse import bacc, mybir

    dt = mybir.dt
    f32, bf16 = dt.float32, dt.bfloat16
    AF = mybir.ActivationFunctionType
    OP = mybir.AluOpType

    nc = bacc.Bacc("TRN2", target_bir_lowering=False, debug=False)

    xT = nc.dram_tensor("xT", [KT, 128, T], bf16, kind="ExternalInput").ap()
    wqk = nc.dram_tensor("wqk", [KT, 128, 2 * C], bf16, kind="ExternalInput").ap()
    wv = nc.dram_tensor("wv", [KT, 128, C], bf16, kind="ExternalInput").ap()
    wpj = nc.dram_tensor("wpj", [KT, 128, C], bf16, kind="ExternalInput").ap()
    bT = nc.dram_tensor("bT", [KT, N, 2 * N], bf16, kind="ExternalInput").ap()
    bqk = nc.dram_tensor("bqk", [128, 2 * KT], f32, kind="ExternalInput").ap()
    outT = nc.dram_tensor("outT", [KT, 128, T], f32, kind="ExternalOutput").ap()

    JROWS = (128, N - 128)  # 128, 69

    with tile.TileContext(nc) as tc:
        from contextlib import ExitStack

        with ExitStack() as ctx:
            cp = ctx.enter_context(tc.tile_pool(name="consts", bufs=1))
            psA = ctx.enter_context(tc.tile_pool(name="psA", bufs=2, space="PSUM"))
            psS = ctx.enter_context(tc.tile_pool(name="psS", bufs=2, space="PSUM"))
            psO = ctx.enter_context(tc.tile_pool(name="psO", bufs=1, space="PSUM"))
            wp = ctx.enter_context(tc.tile_pool(name="work", bufs=2))

            # ---- persistent SBUF tiles; DMAs in consumption order ----
            x_sb = []
            for k in range(KT):
                t_ = cp.tile([128, T], bf16, name=f"x{k}", tag=f"x{k}")
                nc.sync.dma_start(out=t_[:], in_=xT[k])
                x_sb.append(t_)
            wv_sb = []
            for k in range(KT):
                t_ = cp.tile([128, C], bf16, name=f"wv{k}", tag=f"wv{k}")
                nc.gpsimd.dma_start(out=t_[:], in_=wv[k])
                wv_sb.append(t_)
            wqk_sb = []
            for k in range(KT):
                t_ = cp.tile([128, 2 * C], bf16, name=f"wqk{k}", tag=f"wqk{k}")
                nc.gpsimd.dma_start(out=t_[:], in_=wqk[k])
                wqk_sb.append(t_)
            bqk_sb = cp.tile([128, 2 * KT], f32, name="bqk", tag="bqk")
            nc.gpsimd.dma_start(out=bqk_sb[:], in_=bqk[:])
            # bias (hp, jt): both heads of the pair side by side (rows, 2N);
            # loads deferred into the hp loop. proj weights load before D.
            bias_sb = {}
            for hp in range(KT):
                for jt, rows in enumerate(JROWS):
                    bias_sb[(hp, jt)] = cp.tile(
                        [rows, 2 * N], bf16, name=f"bias{hp}_{jt}", tag=f"bias{hp}_{jt}"
                    )
            wpj_sb = [
                cp.tile([128, C], bf16, name=f"wpj{k}", tag=f"wpj{k}")
                for k in range(KT)
            ]

            # qk_sb[0:6] = Q^T tiles (c=0..767), qk_sb[6:12] = K^T tiles
            qk_sb = [
                cp.tile([128, T], bf16, name=f"qk{m}", tag=f"qk{m}")
                for m in range(2 * KT)
            ]
            # V per (batch, jt): plain (rows, 768), head h at cols h*64..
            v_sb = {}
            for b in range(BPC):
                for jt, rows in enumerate(JROWS):
                    v_sb[(b, jt)] = cp.tile(
                        [rows, C], bf16, name=f"v{b}_{jt}", tag=f"v{b}_{jt}"
                    )
            attn_sb = [
                cp.tile([128, T], bf16, name=f"at{m}", tag=f"at{m}") for m in range(KT)
            ]

            # ones columns for the softmax-denominator matmuls, and a ones row
            # for the reciprocal broadcast matmul
            onesc = {}
            for jt, rows in enumerate(JROWS):
                oc = cp.tile([rows, 1], bf16, name=f"onesc{jt}", tag=f"onesc{jt}")
                nc.vector.memset(oc[:], 1.0)
                onesc[jt] = oc
            ones_r = cp.tile([1, 128], bf16, name="ones_r", tag="ones_r")
            nc.vector.memset(ones_r[:], 1.0)

            # ---- Phase B: V in (t, c) layout ----
            for b in range(BPC):
                for jt, rows in enumerate(JROWS):
                    for n2 in range(2):
                        psv = psA.tile([128, TN], f32, tag="psA")
                        for k in range(KT):
                            nc.tensor.matmul(
                                psv[0:rows, 0 : C // 2],
                                x_sb[k][:, b * N + jt * 128 : b * N + jt * 128 + rows],
                                wv_sb[k][:, n2 * (C // 2) : (n2 + 1) * (C // 2)],
                                start=(k == 0),
                                stop=(k == KT - 1),
                            )
                        # v-bias folds into the host-side output constant
                        nc.vector.tensor_copy(
                            v_sb[(b, jt)][0:rows, n2 * (C // 2) : (n2 + 1) * (C // 2)],
                            psv[0:rows, 0 : C // 2],
                        )

            # ---- Phase A (by head-pair) interleaved with Phase C ----
            def emit_proj_tile(mt):
                for nt in range(NT):
                    ps = psA.tile([128, TN], f32, tag="psA")
                    for k in range(KT):
                        nc.tensor.matmul(
                            ps[:],
                            wqk_sb[k][:, mt * 128 : (mt + 1) * 128],
                            x_sb[k][:, nt * TN : (nt + 1) * TN],
                            start=(k == 0),
                            stop=(k == KT - 1),
                        )
                    nc.scalar.activation(
                        qk_sb[mt][:, nt * TN : (nt + 1) * TN],
                        ps[:],
                        AF.Identity,
                        bias=bqk_sb[:, mt : mt + 1],
                    )

            for hp in range(KT):
                for jt, rows in enumerate(JROWS):
                    nc.sync.dma_start(
                        out=bias_sb[(hp, jt)][:],
                        in_=bT[hp, jt * 128 : jt * 128 + rows, :],
                    )
                emit_proj_tile(hp)          # Q^T tile for this head pair
                emit_proj_tile(KT + hp)     # K^T tile
                for b in range(BPC):
                    e_tiles = {}
                    # sums for both heads side by side: (1, 2N)
                    s_ps = psO.tile([1, 2 * N], f32, tag="psSum", bufs=1)
                    for hh in range(2):
                        h = 2 * hp + hh
                        base = 64 * hh
                        for jt, rows in enumerate(JROWS):
                            ps = psS.tile([128, N], f32, tag="psS")
                            nc.tensor.matmul(
                                ps[0:rows, :],
                                qk_sb[KT + hp][
                                    base : base + 64,
                                    b * N + jt * 128 : b * N + jt * 128 + rows,
                                ],
                                qk_sb[hp][base : base + 64, b * N : (b + 1) * N],
                                start=True,
                                stop=True,
                            )
                            eu = wp.tile([128, N], bf16, tag=f"eu{hh}{jt}", bufs=2)
                            nc.scalar.activation(eu[0:rows, :], ps[0:rows, :], AF.Exp)
                            # multiplicative rel-pos bias (exp'd on host)
                            e = wp.tile([128, N], bf16, tag=f"e{hh}{jt}", bufs=2)
                            nc.vector.tensor_mul(
                                e[0:rows, :],
                                eu[0:rows, :],
                                bias_sb[(hp, jt)][
                                    0:rows, hh * N : (hh + 1) * N
                                ],
                            )
                            e_tiles[(hh, jt)] = e
                        # softmax denominators for this head
                        for jt, rows in enumerate(JROWS):
                            nc.tensor.matmul(
                                s_ps[0:1, hh * N : (hh + 1) * N],
                                onesc[jt][0:rows, :],
                                e_tiles[(hh, jt)][0:rows, :],
                                start=(jt == 0),
                                stop=(jt == 1),
                            )
                    r2 = wp.tile([1, 2 * N], bf16, tag="r2", bufs=4)
                    with nc.allow_low_precision(
                        reason="softmax denom reciprocal in bf16 for PE broadcast"
                    ):
                        nc.vector.reciprocal(r2[:], s_ps[:])
                    # broadcast 1/s to all 128 partitions (both heads' halves)
                    rbp = psO.tile([128, 2 * N], f32, tag="psO")
                    nc.tensor.matmul(rbp[:], ones_r[:], r2[:], start=True, stop=True)
                    rb = wp.tile([128, 2 * N], bf16, tag="rb", bufs=2)
                    nc.scalar.activation(rb[:], rbp[:], AF.Copy)
                    for hh in range(2):
                        h = 2 * hp + hh
                        base = 64 * hh
                        po = psS.tile([128, 512], f32, tag="psPo", bufs=2)
                        for jt, rows in enumerate(JROWS):
                            nc.tensor.matmul(
                                po[base : base + 64, 0:N],
                                v_sb[(b, jt)][0:rows, h * 64 : (h + 1) * 64],
                                e_tiles[(hh, jt)][0:rows, :],
                                start=(jt == 0),
                                stop=(jt == 1),
                            )
                        nc.vector.tensor_mul(
                            attn_sb[hp][base : base + 64, b * N : (b + 1) * N],
                            po[base : base + 64, 0:N],
                            rb[base : base + 64, hh * N : (hh + 1) * N],
                        )

            # ---- Phase D: proj -> out^T(c,t); proj bias added on host ----
            for k in range(KT):
                nc.sync.dma_start(out=wpj_sb[k][:], in_=wpj[k])
            for mt in range(KT):
                for nt in range(NT):
                    ps = psA.tile([128, TN], f32, tag="psA")
                    for k in range(KT):
                        nc.tensor.matmul(
                            ps[:],
                            wpj_sb[k][:, mt * 128 : (mt + 1) * 128],
                            attn_sb[k][:, nt * TN : (nt + 1) * TN],
                            start=(k == 0),
                            stop=(k == KT - 1),
                        )
                    ot = wp.tile([128, TN], f32, tag="ot", bufs=3)
                    nc.scalar.activation(ot[:], ps[:], AF.Copy)
                    nc.sync.dma_start(
                        out=outT[mt, :, nt * TN : (nt + 1) * TN], in_=ot[:]
                    )

    if finalize:
        nc.finalize()
    return nc


def _host_prep(x, qkv_w, qkv_b, proj_w, proj_b, rel_table, log_temp, rel_index):
    """Build the per-core input maps (host-side layout prep only)."""
    x = np.asarray(x, np.float32)
    qkv_w = np.asarray(qkv_w, np.float32)
    qkv_b = np.asarray(qkv_b, np.float32)
    proj_w = np.asarray(proj_w, np.float32)
    rel_table = np.asarray(rel_table, np.float32)
    log_temp = np.asarray(log_temp, np.float32)
    rel_index = np.asarray(rel_index)

    temp = np.log1p(np.exp(log_temp.astype(np.float64))).astype(np.float32)  # softplus
    alpha = (SCALE / temp).astype(np.float32)         # (H,) folded into q
    alpha_c = np.repeat(alpha, D)                     # (768,)

    wqkT = qkv_w[0 : 2 * C].T.copy()                  # (768, 1536)
    wqkT[:, 0:C] *= alpha_c[None, :]
    wqk_np = wqkT.reshape(KT, 128, 2 * C).astype(BF16)

    wv_np = qkv_w[2 * C : 3 * C].T.reshape(KT, 128, C).astype(BF16)
    wpj_np = proj_w.T.reshape(KT, 128, C).astype(BF16)

    bq = qkv_b[0:C] * alpha_c
    bk = qkv_b[C : 2 * C]
    bqk_np = np.concatenate([bq, bk]).reshape(2 * KT, 128).T.copy().astype(np.float32)

    # multiplicative bias table: exp((relpos bias)/temp), diag -> 0, CLS -> 1,
    # transposed to (j, i)
    rpb = rel_table[rel_index]                        # (196, 196, H)
    bias = np.zeros((H, N, N), np.float32)
    bias[:, 1:, 1:] = rpb.transpose(2, 0, 1) / temp[:, None, None]
    ebias = np.exp(bias)
    idx = np.arange(1, N)
    ebias[:, idx, idx] = 0.0
    ebT = ebias.transpose(0, 2, 1)                    # (H, j, i)
    # paired layout: (KT, j, 2N) = heads 2hp | 2hp+1 side by side
    bT_np = (
        ebT.reshape(KT, 2, N, N).transpose(0, 2, 1, 3).reshape(KT, N, 2 * N)
    ).astype(BF16).copy()

    in_maps = []
    for c in range(NCORES):
        xc = x[c * BPC : (c + 1) * BPC].reshape(T, C).T  # (768, T)
        xT_np = xc.reshape(KT, 128, T).astype(BF16)
        in_maps.append(
            {
                "xT": xT_np,
                "wqk": wqk_np,
                "wv": wv_np,
                "wpj": wpj_np,
                "bT": bT_np,
                "bqk": bqk_np,
            }
        )
    return in_maps


def _unshard_core(sim, inputs):
    """Dev-only: reconstruct core-0 output from a CoreSim run."""
    proj_b = np.asarray(inputs["proj_b"], np.float32)
    proj_w = np.asarray(inputs["proj_w"], np.float32)
    bv = np.asarray(inputs["qkv_b"], np.float32)[2 * C : 3 * C]
    b_eff = proj_b + proj_w @ bv
    oT = np.asarray(sim.tensor("outT"), np.float32).reshape(C, T)
    return oT.T.reshape(BPC, N, C) + b_eff[None, None, :]


def kernel(**inputs) -> np.ndarray:
    global LAST_RESULTS
    from concourse.bass_utils import run_bass_kernel_spmd

    if "nc" not in _CACHE:
        _CACHE["nc"] = _build()
    nc = _CACHE["nc"]

    in_maps = _host_prep(**inputs)
    try:
        res = run_bass_kernel_spmd(
            nc, in_maps, core_ids=list(range(NCORES)), trace=TRACE
        )
    except ModuleNotFoundError:
        res = run_bass_kernel_spmd(
            nc, in_maps, core_ids=list(range(NCORES)), trace=False
        )
    LAST_RESULTS = res

    # v-bias rides through attention unchanged (rows of attn sum to 1), so
    # its proj image folds into the constant output bias added here
    proj_b = np.asarray(inputs["proj_b"], np.float32)
    proj_w = np.asarray(inputs["proj_w"], np.float32)
    bv = np.asarray(inputs["qkv_b"], np.float32)[2 * C : 3 * C]
    b_eff = proj_b + proj_w @ bv
    outs = []
    for c in range(NCORES):
        oT = np.asarray(res.results[c]["outT"], np.float32).reshape(C, T)
        outs.append(oT.T.reshape(BPC, N, C))
    out = np.concatenate(outs, axis=0) + b_eff[None, None, :]
    return out.astype(np.float32)

